# revision 42
# baseline (speedup 1.0000x reference)
"""Trainium2 Bass kernel for nn_CrossAttentionLayer (sparse cross attention).

Sharding: 8 cores = 4 batches x 2 head-groups. Core c handles batch c//2 and
heads [4*(c%2), 4*(c%2)+4). Each core computes LN + q/k/v projections for its
shard, flash-style masked attention in transposed layout, and a partial
out-projection. Host sums the two per-batch partials and adds bo.

Device algorithm (per core), all matmuls bf16 with fp32 PSUM accumulation:
  xlnT   = transpose(layernorm(x))            (LN gains/biases folded into W/b)
  qT/kT  = W.T @ xlnT   [d, tok]              (per-partition bias via ACT)
  v      = xlnT.T @ Wv  [tok, d]  * kv_mask   (kv_mask folded into v + ones col)
  sT     = kT.T-blocks @ qT-blocks            [k, q] scores, transposed
  pT     = exp(sT * scale) * sparse_mask.T    (ACT exp + DVE mask multiply)
  accT   = [v | kvm].T @ pT                   rows 0-63 = unnormalized out.T,
                                              row 64 = softmax denominator
  aT     = accT[0:64] * (1/denominator)       (broadcast via DMA replicate)
  out    = aT.T-blocks @ Wo-blocks            [q, E] partial, fp32 to HBM
"""

import os

import numpy as np
import ml_dtypes

import bass_rust
import concourse.bass as bass
import concourse.mybir as mybir
import concourse.tile as tile
from concourse import bass_utils
from concourse.masks import make_identity
from concourse.vector_clock import ScopedClock


class _TileContext(tile.TileContext):
    """TileContext whose kernel-tail drain is split into single-wait drains.

    The walrus build in this environment rejects >1 sync-wait on a Drain
    (CTRL_NO struct): "Too many sync wait commands". The stock
    _drain_and_barrier attaches one wait per outstanding semaphore to a
    single Drain; emit one Drain per wait instead.
    """

    def _drain_and_barrier(self, tick_clock, wait_clock):
        drain_inst = self.nc.sync.drain()
        wait_clock.add_sem_waits(
            drain_inst.ins, ScopedClock({None: tick_clock.global_clock})
        )
        si = drain_inst.ins.sync_info
        if si is not None and si.on_wait and len(si.on_wait) > 1:
            waits = list(si.on_wait)
            drain_inst.ins.sync_info = bass_rust.SyncInfo(
                on_wait=[waits[0]], on_update=si.on_update or [])
            for w in waits[1:]:
                extra = self.nc.sync.drain()
                extra.ins.sync_info = bass_rust.SyncInfo(
                    on_wait=[w], on_update=[])

        self.nc.all_engine_barrier()
        assert self.sems is not None
        popped = self.nc._tile_sem_poison_stack.pop()
        assert popped is self._sem_poison
        self.nc.clear_and_free_semaphores(list(self.sems.allocated().values()))
        self.nc.all_engine_barrier()

def _split_sync_waits(nc):
    """Cap every instruction at one sync wait.

    This walrus build rejects instructions carrying more than one sem wait
    ("Too many sync wait commands", setupSyncWait) across several structs
    (Drain, DMACopy, ...). Move excess waits onto no-op instructions placed
    immediately before the offender on the same engine — identical ordering
    semantics, one wait per instruction.
    """
    for f in nc.m.functions:
        for bb in f.blocks:
            insns = bb.instructions
            out = []
            changed = False
            for ins in insns:
                si = ins.sync_info
                if si is not None and si.on_wait and len(si.on_wait) > 1:
                    waits = list(si.on_wait)
                    for w in waits[:-1]:
                        nop = mybir.InstNoOp(
                            name=nc.get_next_instruction_name(),
                            engine=ins.engine,
                            ins=[], outs=[],
                            sync_info=bass_rust.SyncInfo(
                                on_wait=[w], on_update=[]),
                        )
                        out.append(nop)
                    ins.sync_info = bass_rust.SyncInfo(
                        on_wait=[waits[-1]], on_update=si.on_update or [])
                    changed = True
                out.append(ins)
            if changed:
                bb.instructions = out


BF16 = ml_dtypes.bfloat16

E = 512
H = 8
D = 64
T = 2048           # tokens (both query and key side)
P = 128
NT = T // P        # 16 token tiles
EC = E // P        # 4 contraction chunks
HC = 4             # heads per core
MC = 2             # 128-wide chunks of this core's 256 head dims
QC = 4             # 512-wide query chunks
SCALE = float(D) ** -0.5
EPS = 1e-5

_CACHE = {}


def _build(needs_bv: bool, reps: int = 1):
    nc = bass.Bass("TRN2", target_bir_lowering=False, debug=False, num_devices=8)
    f32 = mybir.dt.float32
    bf16 = mybir.dt.bfloat16

    xq = nc.dram_tensor("xq", [T, E], f32, kind="ExternalInput").ap()
    xkv = nc.dram_tensor("xkv", [T, E], f32, kind="ExternalInput").ap()
    wq = nc.dram_tensor("wq", [E, MC * P], bf16, kind="ExternalInput").ap()
    wk = nc.dram_tensor("wk", [E, MC * P], bf16, kind="ExternalInput").ap()
    wv = nc.dram_tensor("wv", [E, MC * P], bf16, kind="ExternalInput").ap()
    wo = nc.dram_tensor("wo", [MC * P, E], bf16, kind="ExternalInput").ap()
    bqd = nc.dram_tensor("bq", [P, MC], f32, kind="ExternalInput").ap()
    bkd = nc.dram_tensor("bk", [P, MC], f32, kind="ExternalInput").ap()
    kvmd = nc.dram_tensor("kvm", [P, NT], f32, kind="ExternalInput").ap()
    mtd = nc.dram_tensor("mt", [T, T], bf16, kind="ExternalInput").ap()
    if needs_bv:
        bvd = nc.dram_tensor("bv", [1, MC * P], bf16, kind="ExternalInput").ap()
    outd = nc.dram_tensor("out", [T, E], f32, kind="ExternalOutput").ap()

    with _TileContext(nc) as tc:
        with (
            tc.tile_pool(name="persist", bufs=1) as pp,
            tc.tile_pool(name="xs", bufs=4) as xpool,
            tc.tile_pool(name="work", bufs=4) as wk_pool,
            tc.tile_pool(name="scratch", bufs=4) as scratch,
            tc.tile_pool(name="psA", bufs=2, space="PSUM") as psA,
            tc.tile_pool(name="psS", bufs=2, space="PSUM") as psS,
            tc.tile_pool(name="psC", bufs=2, space="PSUM") as psC,
        ):
            # ---- persistent SBUF tensors ----
            # xlnT / qT / aT are split into per-group tensors so downstream
            # consumers unblock as soon as their group is written (Tile tracks
            # dependencies per tile, so monolithic tensors serialize phases).
            wq_sb = pp.tile([P, EC, MC * P], bf16, tag="wq")
            wk_sb = pp.tile([P, EC, MC * P], bf16, tag="wk")
            wv_sb = pp.tile([P, EC, MC * P], bf16, tag="wv")
            wo_sb = pp.tile([P, MC, E], bf16, tag="wo")
            bq_sb = pp.tile([P, MC], f32, tag="bq")
            bk_sb = pp.tile([P, MC], f32, tag="bk")
            kvm_sb = pp.tile([P, NT], f32, tag="kvm")
            mt_sb = pp.tile([P, NT, T], bf16, tag="mt")
            xlnq_g = [pp.tile([P, 4, EC, P], bf16, tag=f"xlnq{g}",
                              name=f"xlnq{g}") for g in range(4)]
            xlnkv_g = [pp.tile([P, 4, EC, P], bf16, tag=f"xlnkv{g}",
                               name=f"xlnkv{g}") for g in range(4)]
            qT_g = [pp.tile([P, MC, 512], bf16, tag=f"qT{g}", name=f"qT{g}")
                    for g in range(4)]
            kT_sb = pp.tile([P, MC, T], bf16, tag="kT")
            v_sb = pp.tile([P, NT, HC * (D + 1)], bf16, tag="v")
            aT_g = [pp.tile([P, MC, 512], bf16, tag=f"aT{g}", name=f"aT{g}")
                    for g in range(4)]
            sums_g = [pp.tile([P, HC * 4], f32, tag=f"sums{g}",
                              name=f"sums{g}") for g in range(QC)]
            rsp_g = [pp.tile([P, HC * 4], bf16, tag=f"rsp{g}",
                             name=f"rsp{g}") for g in range(QC)]
            rs_flat_g = [pp.tile([1, HC, 512], bf16, tag=f"rsflat{g}",
                                 name=f"rsflat{g}") for g in range(QC)]
            if needs_bv:
                bv_sb = pp.tile([1, MC * P], bf16, tag="bv")
                ones_sb = pp.tile([1, P], bf16, tag="ones")

            eps_sb = pp.tile([P, 1], f32, tag="eps")
            nc.vector.memset(eps_sb[:], EPS)
            if needs_bv:
                nc.sync.dma_start(bv_sb[:], bvd)
                nc.vector.memset(ones_sb[:], 1.0)
            ident = pp.tile([P, P], bf16, tag="ident")
            make_identity(nc, ident[:])
            ones1 = pp.tile([1, D], bf16, tag="ones1")
            nc.vector.memset(ones1[:], 1.0)
            mtr = mtd.rearrange("(c p) q -> p c q", p=P)

            def ln_group(src, dstT, g):
                """LN 4 token tiles of src into dstT ([P, 4, EC, P])."""
                for tp in range(2):
                    # one [128, 1024] bf16 psum tile (1 bank) holds the
                    # transposes of a PAIR of token tiles; one wide copy out
                    ptr = psA.tile([P, 2, E], bf16, tag="p512")
                    for ti2 in range(2):
                        ti = tp * 2 + ti2
                        t = g * 4 + ti
                        xt = xpool.tile([P, E], f32, tag="x")
                        nc.sync.dma_start(xt[:], src[t * P:(t + 1) * P, :])
                        stats = scratch.tile([P, 6], f32, tag="bnstats")
                        mv = scratch.tile([P, 2], f32, tag="bnmv")
                        nc.vector.bn_stats(stats[:], xt[:])
                        nc.vector.bn_aggr(mv[:], stats[:])
                        sig = scratch.tile([P, 1], f32, tag="sig")
                        nc.scalar.activation(
                            sig[:], mv[:, 1:2],
                            mybir.ActivationFunctionType.Sqrt,
                            bias=eps_sb[:])
                        rsig = scratch.tile([P, 1], f32, tag="rsig")
                        nc.vector.reciprocal(rsig[:], sig[:])
                        xln = wk_pool.tile([P, E], bf16, tag="xln")
                        nc.vector.tensor_scalar(
                            xln[:], xt[:], mv[:, 0:1], rsig[:],
                            mybir.AluOpType.subtract, mybir.AluOpType.mult)
                        for c in range(EC):
                            nc.tensor.transpose(
                                ptr[:, ti2, c * P:(c + 1) * P],
                                xln[:, c * P:(c + 1) * P], ident[:])
                    nc.vector.tensor_copy(
                        dstT[:, 2 * tp:2 * tp + 2],
                        ptr[:].rearrange("p u (c n) -> p u c n", n=P))

            def kproj_group(g):
                for mc in range(MC):
                    ps = psA.tile([P, 512], mybir.dt.float32, tag="p512")
                    for c in range(EC):
                        nc.tensor.matmul(
                            ps[:],
                            lhsT=wk_sb[:, c, mc * P:(mc + 1) * P],
                            rhs=xlnkv_g[g][:, :, c, :],
                            start=(c == 0), stop=(c == EC - 1))
                    nc.scalar.activation(
                        kT_sb[:, mc, g * 512:(g + 1) * 512], ps[:],
                        mybir.ActivationFunctionType.Identity,
                        bias=bk_sb[:, mc:mc + 1])

            def vproj_group(g):
                for ti in range(4):
                    t = g * 4 + ti
                    ps = psA.tile([P, MC * P], mybir.dt.float32, tag="p512")
                    for c in range(EC):
                        nc.tensor.matmul(
                            ps[:],
                            lhsT=xlnkv_g[g][:, ti, c, :],
                            rhs=wv_sb[:, c, :],
                            start=(c == 0),
                            stop=(c == EC - 1 and not needs_bv))
                    if needs_bv:
                        nc.tensor.matmul(
                            ps[:], lhsT=ones_sb[:], rhs=bv_sb[:],
                            start=False, stop=True)
                    vd = v_sb[:, t].rearrange("p (h d) -> p h d", d=D + 1)
                    nc.vector.tensor_scalar(
                        vd[:, :, 0:D], ps.rearrange("p (h d) -> p h d", d=D),
                        kvm_sb[:, t:t + 1], None, mybir.AluOpType.mult)
                    nc.vector.tensor_copy(
                        vd[:, :, D], kvm_sb[:, t:t + 1].to_broadcast((P, HC)))

            def qproj_group(g):
                for mc in range(MC):
                    ps = psA.tile([P, 512], mybir.dt.float32, tag="p512")
                    for c in range(EC):
                        nc.tensor.matmul(
                            ps[:],
                            lhsT=wq_sb[:, c, mc * P:(mc + 1) * P],
                            rhs=xlnq_g[g][:, :, c, :],
                            start=(c == 0), stop=(c == EC - 1))
                    nc.scalar.activation(
                        qT_g[g][:, mc, :], ps[:],
                        mybir.ActivationFunctionType.Identity,
                        bias=bq_sb[:, mc:mc + 1])

            # kv side first (attention needs all of kT/v); mask chunks stream
            # in behind the x loads; q-side groups unblock attention per qc.
            rep_ctx = tc.For_i(0, reps, 1) if reps > 1 else None
            if rep_ctx is not None:
                rep_ctx.__enter__()
            for g in range(4):
                ln_group(xkv, xlnkv_g[g], g)
                if g == 0:
                    # weights land behind the first x tiles on the DMA queues
                    nc.sync.dma_start(
                        wk_sb[:], wk.rearrange("(c p) n -> p c n", p=P))
                    nc.sync.dma_start(
                        wv_sb[:], wv.rearrange("(c p) n -> p c n", p=P))
                    nc.sync.dma_start(
                        wq_sb[:], wq.rearrange("(c p) n -> p c n", p=P))
                    nc.sync.dma_start(
                        wo_sb[:], wo.rearrange("(c p) n -> p c n", p=P))
                    nc.sync.dma_start(bq_sb[:], bqd)
                    nc.sync.dma_start(bk_sb[:], bkd)
                    nc.sync.dma_start(kvm_sb[:], kvmd)
                kproj_group(g)
                vproj_group(g)
                for c in range(4):
                    nc.sync.dma_start(mt_sb[:, 4 * g + c], mtr[:, 4 * g + c])
            for g in range(4):
                ln_group(xq, xlnq_g[g], g)
                qproj_group(g)

            # ---- attention: qc outer so normalize+out_proj overlap ----
            for qc in range(QC):
                for h in range(HC):
                    mc = h // 2
                    po = (h % 2) * D
                    acc = psC.tile([P, 512], mybir.dt.float32, tag="acc")
                    for kcp in range(NT // 2):
                        sp = psS.tile([P, 2, 512], mybir.dt.float32, tag="sp")
                        for j in range(2):
                            kc = 2 * kcp + j
                            nc.tensor.matmul(
                                sp[:, j],
                                lhsT=kT_sb[po:po + D, mc, kc * P:(kc + 1) * P],
                                rhs=qT_g[qc][po:po + D, mc, :],
                                start=True, stop=True)
                        pT = wk_pool.tile([P, 2, 512], bf16, tag="pT")
                        nc.scalar.activation(
                            pT[:], sp[:], mybir.ActivationFunctionType.Exp,
                            scale=SCALE)
                        nc.vector.tensor_tensor(
                            pT[:], pT[:],
                            mt_sb[:, 2 * kcp:2 * kcp + 2,
                                  qc * 512:(qc + 1) * 512],
                            mybir.AluOpType.mult)
                        for j in range(2):
                            kc = 2 * kcp + j
                            nc.tensor.matmul(
                                acc[:D + 1],
                                lhsT=v_sb[:, kc, h * (D + 1):(h + 1) * (D + 1)],
                                rhs=pT[:, j],
                                start=(kcp == 0 and j == 0),
                                stop=(kcp == NT // 2 - 1 and j == 1))
                    stage = wk_pool.tile([P, 512], f32, tag="sumstage")
                    nc.vector.tensor_copy(stage[D:D + 1, :], acc[D:D + 1, :])
                    # gather this head's denominators into [P, 4] of sums_g
                    # (DMA streams element-linearly: q index = p*4 + f)
                    nc.sync.dma_start(
                        sums_g[qc][:, h * 4:(h + 1) * 4], stage[D:D + 1, :])
                    nc.vector.tensor_copy(
                        aT_g[qc][po:po + D, mc, :], acc[:D])

                    if h % 2 == 1:
                        # this head pair (chunk mc) is complete: normalize it
                        # now so the chain overlaps the remaining heads
                        sl = slice(8 * mc, 8 * mc + 8)
                        nc.vector.tensor_scalar(
                            sums_g[qc][:, sl], sums_g[qc][:, sl], 1e-30, None,
                            mybir.AluOpType.add)
                        rsp = rsp_g[qc]
                        rsf = scratch.tile([P, 8], f32, tag="rsf")
                        nc.vector.reciprocal(rsf[:], sums_g[qc][:, sl])
                        nc.vector.tensor_copy(rsp[:, sl], rsf[:])
                        for hh in (2 * mc, 2 * mc + 1):
                            nc.sync.dma_start(
                                rs_flat_g[qc][:, hh, :],
                                rsp[:, hh * 4:(hh + 1) * 4])
                        rb = psA.tile([P, 512], mybir.dt.float32, tag="p512")
                        nc.tensor.matmul(
                            rb[0:D], lhsT=ones1[:],
                            rhs=rs_flat_g[qc][:, 2 * mc, :],
                            start=True, stop=True)
                        nc.tensor.matmul(
                            rb[D:2 * D], lhsT=ones1[:],
                            rhs=rs_flat_g[qc][:, 2 * mc + 1, :],
                            start=True, stop=True)
                        nc.vector.tensor_tensor(
                            aT_g[qc][:, mc, :], aT_g[qc][:, mc, :],
                            rb[:], mybir.AluOpType.mult)

                # out projection for this qc's 4 token tiles
                for ti in range(4):
                    t = qc * 4 + ti
                    ps = psA.tile([P, E], mybir.dt.float32, tag="p512")
                    for mc in range(MC):
                        nc.tensor.matmul(
                            ps[:],
                            lhsT=aT_g[qc][:, mc, ti * P:(ti + 1) * P],
                            rhs=wo_sb[:, mc, :],
                            start=(mc == 0), stop=(mc == MC - 1))
                    osb = wk_pool.tile([P, E], f32, tag="osb")
                    nc.vector.tensor_copy(osb[:], ps[:])
                    nc.sync.dma_start(outd[t * P:(t + 1) * P, :], osb[:])

            if rep_ctx is not None:
                rep_ctx.__exit__(None, None, None)

    _split_sync_waits(nc)
    return nc


def _get_nc(needs_bv: bool, reps: int = 1):
    key = ("nc", needs_bv, reps)
    if key not in _CACHE:
        _CACHE[key] = _build(needs_bv, reps)
    return _CACHE[key]


def kernel(query, key_value, kv_mask, sparse_mask,
           ln_q_g, ln_q_b, ln_kv_g, ln_kv_b,
           Wq, bq, Wk, bk, Wv, bv, Wo, bo):
    query = np.asarray(query, np.float32)
    key_value = np.asarray(key_value, np.float32)
    kv_mask = np.asarray(kv_mask)
    sparse_mask = np.asarray(sparse_mask)
    B = query.shape[0]

    # Fold LN gain/bias into the projection weights (exact algebra):
    # (x_ln*g + b) @ W + c  ==  x_ln @ (g[:,None]*W) + (b@W + c)
    Wq_g = np.asarray(ln_q_g, np.float32)[:, None] * np.asarray(Wq, np.float32)
    Wk_g = np.asarray(ln_kv_g, np.float32)[:, None] * np.asarray(Wk, np.float32)
    Wv_g = np.asarray(ln_kv_g, np.float32)[:, None] * np.asarray(Wv, np.float32)
    bq_e = np.asarray(ln_q_b, np.float32) @ np.asarray(Wq, np.float32) + bq
    bk_e = np.asarray(ln_kv_b, np.float32) @ np.asarray(Wk, np.float32) + bk
    bv_e = np.asarray(ln_kv_b, np.float32) @ np.asarray(Wv, np.float32) + bv

    needs_bv = bool(np.any(bv_e != 0.0))
    reps = int(os.environ.get("KERNEL_REPS", "1"))
    nc = _get_nc(needs_bv, reps)

    in_maps = []
    for c in range(8):
        b, hg = c // 2, c % 2
        hs = slice(hg * MC * P, (hg + 1) * MC * P)
        m = {
            "xq": np.ascontiguousarray(query[b]),
            "xkv": np.ascontiguousarray(key_value[b]),
            "wq": np.ascontiguousarray(Wq_g[:, hs]).astype(BF16),
            "wk": np.ascontiguousarray(Wk_g[:, hs]).astype(BF16),
            "wv": np.ascontiguousarray(Wv_g[:, hs]).astype(BF16),
            "wo": np.ascontiguousarray(np.asarray(Wo, np.float32)[hs, :]).astype(BF16),
            "bq": np.ascontiguousarray(bq_e[hs].reshape(MC, P).T),
            "bk": np.ascontiguousarray(bk_e[hs].reshape(MC, P).T),
            "kvm": np.ascontiguousarray(
                kv_mask[b].astype(np.float32).reshape(NT, P).T),
            "mt": np.ascontiguousarray(sparse_mask[b].T).astype(BF16),
        }
        if needs_bv:
            m["bv"] = bv_e[hs].astype(BF16).reshape(1, MC * P)
        in_maps.append(m)

    res = bass_utils.run_bass_kernel_spmd(
        nc, in_maps, core_ids=list(range(8)),
        trace=bool(os.environ.get("KERNEL_TRACE")))
    globals()["LAST_RESULTS"] = res

    bo_f = np.asarray(bo, np.float32)
    out = np.empty((B, T, E), np.float32)
    for b in range(B):
        out[b] = res.results[2 * b]["out"] + res.results[2 * b + 1]["out"] + bo_f
    return out


# revision 48
# speedup vs baseline: 1.2313x; 1.2313x over previous
"""Trainium2 Bass kernel for nn_CrossAttentionLayer (sparse cross attention).

Sharding: 8 cores = 4 batches x 2 head-groups. Core c handles batch c//2 and
heads [4*(c%2), 4*(c%2)+4). Each core computes LN + q/k/v projections for its
shard, flash-style masked attention in transposed layout, and a partial
out-projection. Host sums the two per-batch partials and adds bo.

Device algorithm (per core), all matmuls bf16 with fp32 PSUM accumulation:
  xlnT   = transpose(layernorm(x))            (LN gains/biases folded into W/b)
  qT/kT  = W.T @ xlnT   [d, tok]              (per-partition bias via ACT)
  v      = xlnT.T @ Wv  [tok, d]  * kv_mask   (kv_mask folded into v + ones col)
  sT     = kT.T-blocks @ qT-blocks            [k, q] scores, transposed
  pT     = exp(sT * scale) * sparse_mask.T    (ACT exp + DVE mask multiply)
  accT   = [v | kvm].T @ pT                   rows 0-63 = unnormalized out.T,
                                              row 64 = softmax denominator
  aT     = accT[0:64] * (1/denominator)       (broadcast via DMA replicate)
  out    = aT.T-blocks @ Wo-blocks            [q, E] partial, fp32 to HBM
"""

import os

import numpy as np
import ml_dtypes

import bass_rust
import concourse.bass as bass
import concourse.mybir as mybir
import concourse.tile as tile
from concourse import bass_utils
from concourse.masks import make_identity
from concourse.vector_clock import ScopedClock


class _TileContext(tile.TileContext):
    """TileContext whose kernel-tail drain is split into single-wait drains.

    The walrus build in this environment rejects >1 sync-wait on a Drain
    (CTRL_NO struct): "Too many sync wait commands". The stock
    _drain_and_barrier attaches one wait per outstanding semaphore to a
    single Drain; emit one Drain per wait instead.
    """

    def _drain_and_barrier(self, tick_clock, wait_clock):
        drain_inst = self.nc.sync.drain()
        wait_clock.add_sem_waits(
            drain_inst.ins, ScopedClock({None: tick_clock.global_clock})
        )
        si = drain_inst.ins.sync_info
        if si is not None and si.on_wait and len(si.on_wait) > 1:
            waits = list(si.on_wait)
            drain_inst.ins.sync_info = bass_rust.SyncInfo(
                on_wait=[waits[0]], on_update=si.on_update or [])
            for w in waits[1:]:
                extra = self.nc.sync.drain()
                extra.ins.sync_info = bass_rust.SyncInfo(
                    on_wait=[w], on_update=[])

        self.nc.all_engine_barrier()
        assert self.sems is not None
        popped = self.nc._tile_sem_poison_stack.pop()
        assert popped is self._sem_poison
        self.nc.clear_and_free_semaphores(list(self.sems.allocated().values()))
        self.nc.all_engine_barrier()

def _split_sync_waits(nc):
    """Cap every instruction at one sync wait.

    This walrus build rejects instructions carrying more than one sem wait
    ("Too many sync wait commands", setupSyncWait) across several structs
    (Drain, DMACopy, ...). Move excess waits onto no-op instructions placed
    immediately before the offender on the same engine — identical ordering
    semantics, one wait per instruction.
    """
    for f in nc.m.functions:
        for bb in f.blocks:
            insns = bb.instructions
            out = []
            changed = False
            for ins in insns:
                si = ins.sync_info
                if si is not None and si.on_wait and len(si.on_wait) > 1:
                    waits = list(si.on_wait)
                    for w in waits[:-1]:
                        nop = mybir.InstNoOp(
                            name=nc.get_next_instruction_name(),
                            engine=ins.engine,
                            ins=[], outs=[],
                            sync_info=bass_rust.SyncInfo(
                                on_wait=[w], on_update=[]),
                        )
                        out.append(nop)
                    ins.sync_info = bass_rust.SyncInfo(
                        on_wait=[waits[-1]], on_update=si.on_update or [])
                    changed = True
                out.append(ins)
            if changed:
                bb.instructions = out


BF16 = ml_dtypes.bfloat16

E = 512
H = 8
D = 64
T = 2048           # tokens (both query and key side)
P = 128
NT = T // P        # 16 token tiles
EC = E // P        # 4 contraction chunks
HC = 4             # heads per core
MC = 2             # 128-wide chunks of this core's 256 head dims
QC = 4             # 512-wide query chunks
SCALE = float(D) ** -0.5
EPS = 1e-5

_CACHE = {}


def _build(needs_bv: bool, reps: int = 1):
    nc = bass.Bass("TRN2", target_bir_lowering=False, debug=False, num_devices=8)
    f32 = mybir.dt.float32
    bf16 = mybir.dt.bfloat16

    xq = nc.dram_tensor("xq", [T, E], f32, kind="ExternalInput").ap()
    xkv = nc.dram_tensor("xkv", [T, E], f32, kind="ExternalInput").ap()
    wq = nc.dram_tensor("wq", [E, MC * P], bf16, kind="ExternalInput").ap()
    wk = nc.dram_tensor("wk", [E, MC * P], bf16, kind="ExternalInput").ap()
    wv = nc.dram_tensor("wv", [E, MC * P], bf16, kind="ExternalInput").ap()
    wo = nc.dram_tensor("wo", [MC * P, E], bf16, kind="ExternalInput").ap()
    bqd = nc.dram_tensor("bq", [P, MC], f32, kind="ExternalInput").ap()
    bkd = nc.dram_tensor("bk", [P, MC], f32, kind="ExternalInput").ap()
    kvmd = nc.dram_tensor("kvm", [P, NT], f32, kind="ExternalInput").ap()
    mtd = nc.dram_tensor("mt", [T, T], bf16, kind="ExternalInput").ap()
    if needs_bv:
        bvd = nc.dram_tensor("bv", [1, MC * P], bf16, kind="ExternalInput").ap()
    outd = nc.dram_tensor("out", [T, E], f32, kind="ExternalOutput").ap()

    with _TileContext(nc) as tc:
        with (
            tc.tile_pool(name="persist", bufs=1) as pp,
            tc.tile_pool(name="xs", bufs=4) as xpool,
            tc.tile_pool(name="work", bufs=4) as wk_pool,
            tc.tile_pool(name="scratch", bufs=4) as scratch,
            tc.tile_pool(name="psA", bufs=2, space="PSUM") as psA,
            tc.tile_pool(name="psS", bufs=2, space="PSUM") as psS,
            tc.tile_pool(name="psC", bufs=2, space="PSUM") as psC,
        ):
            # ---- persistent SBUF tensors ----
            # xlnT / qT / aT are split into per-group tensors so downstream
            # consumers unblock as soon as their group is written (Tile tracks
            # dependencies per tile, so monolithic tensors serialize phases).
            wq_sb = pp.tile([P, EC, MC * P], bf16, tag="wq")
            wk_sb = pp.tile([P, EC, MC * P], bf16, tag="wk")
            wv_sb = pp.tile([P, EC, MC * P], bf16, tag="wv")
            wo_sb = pp.tile([P, MC, E], bf16, tag="wo")
            bq_sb = pp.tile([P, MC], f32, tag="bq")
            bk_sb = pp.tile([P, MC], f32, tag="bk")
            kvm_sb = pp.tile([P, NT], f32, tag="kvm")
            mt_gt = [pp.tile([P, 4, T], bf16, tag=f"mt{g}", name=f"mt{g}")
                     for g in range(4)]
            xlnq_g = [pp.tile([P, 4, EC, P], bf16, tag=f"xlnq{g}",
                              name=f"xlnq{g}") for g in range(4)]
            xlnkv_g = [pp.tile([P, 4, EC, P], bf16, tag=f"xlnkv{g}",
                               name=f"xlnkv{g}") for g in range(4)]
            qT_g = [pp.tile([P, MC, 512], bf16, tag=f"qT{g}", name=f"qT{g}")
                    for g in range(4)]
            kT_gt = [pp.tile([P, MC, 512], bf16, tag=f"kT{g}", name=f"kT{g}")
                     for g in range(4)]
            v_gt = [pp.tile([P, 4, HC * (D + 1)], bf16, tag=f"v{g}",
                            name=f"v{g}") for g in range(4)]
            aT_g = [pp.tile([P, MC, 512], bf16, tag=f"aT{g}", name=f"aT{g}")
                    for g in range(4)]
            sums_g = [pp.tile([P, HC * 4], f32, tag=f"sums{g}",
                              name=f"sums{g}") for g in range(QC)]
            rsp_g = [pp.tile([P, HC * 4], bf16, tag=f"rsp{g}",
                             name=f"rsp{g}") for g in range(QC)]
            rs_flat_g = [pp.tile([1, HC, 512], bf16, tag=f"rsflat{g}",
                                 name=f"rsflat{g}") for g in range(QC)]
            if needs_bv:
                bv_sb = pp.tile([1, MC * P], bf16, tag="bv")
                ones_sb = pp.tile([1, P], bf16, tag="ones")

            eps_sb = pp.tile([P, 1], f32, tag="eps")
            nc.vector.memset(eps_sb[:], EPS)
            if needs_bv:
                nc.sync.dma_start(bv_sb[:], bvd)
                nc.vector.memset(ones_sb[:], 1.0)
            ident = pp.tile([P, P], bf16, tag="ident")
            make_identity(nc, ident[:])
            ones1 = pp.tile([1, D], bf16, tag="ones1")
            nc.vector.memset(ones1[:], 1.0)
            mtr = mtd.rearrange("(c p) q -> p c q", p=P)

            def ln_group(src, dstT, g):
                """LN 4 token tiles of src into dstT ([P, 4, EC, P])."""
                for tp in range(2):
                    # one [128, 1024] bf16 psum tile (1 bank) holds the
                    # transposes of a PAIR of token tiles; one wide copy out
                    ptr = psA.tile([P, 2, E], bf16, tag="p512")
                    for ti2 in range(2):
                        ti = tp * 2 + ti2
                        t = g * 4 + ti
                        xt = xpool.tile([P, E], f32, tag="x")
                        nc.sync.dma_start(xt[:], src[t * P:(t + 1) * P, :])
                        stats = scratch.tile([P, 6], f32, tag="bnstats")
                        mv = scratch.tile([P, 2], f32, tag="bnmv")
                        nc.vector.bn_stats(stats[:], xt[:])
                        nc.vector.bn_aggr(mv[:], stats[:])
                        sig = scratch.tile([P, 1], f32, tag="sig")
                        nc.scalar.activation(
                            sig[:], mv[:, 1:2],
                            mybir.ActivationFunctionType.Sqrt,
                            bias=eps_sb[:])
                        rsig = scratch.tile([P, 1], f32, tag="rsig")
                        nc.vector.reciprocal(rsig[:], sig[:])
                        xln = wk_pool.tile([P, E], bf16, tag="xln")
                        nc.vector.tensor_scalar(
                            xln[:], xt[:], mv[:, 0:1], rsig[:],
                            mybir.AluOpType.subtract, mybir.AluOpType.mult)
                        for c in range(EC):
                            nc.tensor.transpose(
                                ptr[:, ti2, c * P:(c + 1) * P],
                                xln[:, c * P:(c + 1) * P], ident[:])
                    nc.vector.tensor_copy(
                        dstT[:, 2 * tp:2 * tp + 2],
                        ptr[:].rearrange("p u (c n) -> p u c n", n=P))

            def kproj_group(g):
                for mc in range(MC):
                    ps = psA.tile([P, 512], mybir.dt.float32, tag="p512")
                    for c in range(EC):
                        nc.tensor.matmul(
                            ps[:],
                            lhsT=wk_sb[:, c, mc * P:(mc + 1) * P],
                            rhs=xlnkv_g[g][:, :, c, :],
                            start=(c == 0), stop=(c == EC - 1))
                    nc.scalar.activation(
                        kT_gt[g][:, mc, :], ps[:],
                        mybir.ActivationFunctionType.Identity,
                        bias=bk_sb[:, mc:mc + 1])

            def vproj_group(g):
                for ti in range(4):
                    t = g * 4 + ti
                    ps = psA.tile([P, MC * P], mybir.dt.float32, tag="p512")
                    for c in range(EC):
                        nc.tensor.matmul(
                            ps[:],
                            lhsT=xlnkv_g[g][:, ti, c, :],
                            rhs=wv_sb[:, c, :],
                            start=(c == 0),
                            stop=(c == EC - 1 and not needs_bv))
                    if needs_bv:
                        nc.tensor.matmul(
                            ps[:], lhsT=ones_sb[:], rhs=bv_sb[:],
                            start=False, stop=True)
                    vd = v_gt[g][:, ti].rearrange("p (h d) -> p h d", d=D + 1)
                    nc.vector.tensor_scalar(
                        vd[:, :, 0:D], ps.rearrange("p (h d) -> p h d", d=D),
                        kvm_sb[:, t:t + 1], None, mybir.AluOpType.mult)
                    nc.vector.tensor_copy(
                        vd[:, :, D], kvm_sb[:, t:t + 1].to_broadcast((P, HC)))

            def qproj_group(g):
                for mc in range(MC):
                    ps = psA.tile([P, 512], mybir.dt.float32, tag="p512")
                    for c in range(EC):
                        nc.tensor.matmul(
                            ps[:],
                            lhsT=wq_sb[:, c, mc * P:(mc + 1) * P],
                            rhs=xlnq_g[g][:, :, c, :],
                            start=(c == 0), stop=(c == EC - 1))
                    nc.scalar.activation(
                        qT_g[g][:, mc, :], ps[:],
                        mybir.ActivationFunctionType.Identity,
                        bias=bq_sb[:, mc:mc + 1])

            # kv side first (attention needs all of kT/v); mask chunks stream
            # in behind the x loads; q-side groups unblock attention per qc.
            rep_ctx = tc.For_i(0, reps, 1) if reps > 1 else None
            if rep_ctx is not None:
                rep_ctx.__enter__()
            for g in range(4):
                ln_group(xkv, xlnkv_g[g], g)
                if g == 0:
                    # weights land behind the first x tiles on the DMA queues
                    nc.sync.dma_start(
                        wk_sb[:], wk.rearrange("(c p) n -> p c n", p=P))
                    nc.sync.dma_start(
                        wv_sb[:], wv.rearrange("(c p) n -> p c n", p=P))
                    nc.sync.dma_start(
                        wq_sb[:], wq.rearrange("(c p) n -> p c n", p=P))
                    nc.sync.dma_start(
                        wo_sb[:], wo.rearrange("(c p) n -> p c n", p=P))
                    nc.sync.dma_start(bq_sb[:], bqd)
                    nc.sync.dma_start(bk_sb[:], bkd)
                    nc.sync.dma_start(kvm_sb[:], kvmd)
                kproj_group(g)
                vproj_group(g)
                for c in range(4):
                    nc.sync.dma_start(mt_gt[g][:, c], mtr[:, 4 * g + c])
                if g == 1:
                    # q group 0 early: attention (qc=0) starts on k groups
                    # 0-1 while kv groups 2-3 are still in layernorm
                    ln_group(xq, xlnq_g[0], 0)
                    qproj_group(0)
            for g in range(1, 4):
                ln_group(xq, xlnq_g[g], g)
                qproj_group(g)

            # ---- attention: qc outer so normalize+out_proj overlap ----
            for qc in range(QC):
                for h in range(HC):
                    mc = h // 2
                    po = (h % 2) * D
                    acc = psC.tile([P, 512], mybir.dt.float32, tag="acc")
                    for kcp in range(NT // 2):
                        sp = psS.tile([P, 2, 512], mybir.dt.float32, tag="sp")
                        kg = kcp // 2          # k group (4 k-chunks each)
                        ko = (2 * kcp) % 4     # chunk offset inside group
                        for j in range(2):
                            nc.tensor.matmul(
                                sp[:, j],
                                lhsT=kT_gt[kg][po:po + D, mc,
                                               (ko + j) * P:(ko + j + 1) * P],
                                rhs=qT_g[qc][po:po + D, mc, :],
                                start=True, stop=True)
                        pT = wk_pool.tile([P, 2, 512], bf16, tag="pT")
                        nc.scalar.activation(
                            pT[:], sp[:], mybir.ActivationFunctionType.Exp,
                            scale=SCALE)
                        nc.vector.tensor_tensor(
                            pT[:], pT[:],
                            mt_gt[kg][:, ko:ko + 2, qc * 512:(qc + 1) * 512],
                            mybir.AluOpType.mult)
                        for j in range(2):
                            nc.tensor.matmul(
                                acc[:D + 1],
                                lhsT=v_gt[kg][:, ko + j,
                                              h * (D + 1):(h + 1) * (D + 1)],
                                rhs=pT[:, j],
                                start=(kcp == 0 and j == 0),
                                stop=(kcp == NT // 2 - 1 and j == 1))
                    stage = wk_pool.tile([P, 512], f32, tag="sumstage")
                    nc.vector.tensor_copy(stage[D:D + 1, :], acc[D:D + 1, :])
                    # gather this head's denominators into [P, 4] of sums_g
                    # (DMA streams element-linearly: q index = p*4 + f)
                    nc.sync.dma_start(
                        sums_g[qc][:, h * 4:(h + 1) * 4], stage[D:D + 1, :])
                    nc.vector.tensor_copy(
                        aT_g[qc][po:po + D, mc, :], acc[:D])

                    if h % 2 == 1:
                        # this head pair (chunk mc) is complete: normalize it
                        # now so the chain overlaps the remaining heads
                        sl = slice(8 * mc, 8 * mc + 8)
                        nc.vector.tensor_scalar(
                            sums_g[qc][:, sl], sums_g[qc][:, sl], 1e-30, None,
                            mybir.AluOpType.add)
                        rsp = rsp_g[qc]
                        rsf = scratch.tile([P, 8], f32, tag="rsf")
                        nc.vector.reciprocal(rsf[:], sums_g[qc][:, sl])
                        nc.vector.tensor_copy(rsp[:, sl], rsf[:])
                        for hh in (2 * mc, 2 * mc + 1):
                            nc.sync.dma_start(
                                rs_flat_g[qc][:, hh, :],
                                rsp[:, hh * 4:(hh + 1) * 4])
                        rb = psA.tile([P, 512], mybir.dt.float32, tag="p512")
                        nc.tensor.matmul(
                            rb[0:D], lhsT=ones1[:],
                            rhs=rs_flat_g[qc][:, 2 * mc, :],
                            start=True, stop=True)
                        nc.tensor.matmul(
                            rb[D:2 * D], lhsT=ones1[:],
                            rhs=rs_flat_g[qc][:, 2 * mc + 1, :],
                            start=True, stop=True)
                        nc.vector.tensor_tensor(
                            aT_g[qc][:, mc, :], aT_g[qc][:, mc, :],
                            rb[:], mybir.AluOpType.mult)

                # out projection for this qc's 4 token tiles
                for ti in range(4):
                    t = qc * 4 + ti
                    ps = psA.tile([P, E], mybir.dt.float32, tag="p512")
                    for mc in range(MC):
                        nc.tensor.matmul(
                            ps[:],
                            lhsT=aT_g[qc][:, mc, ti * P:(ti + 1) * P],
                            rhs=wo_sb[:, mc, :],
                            start=(mc == 0), stop=(mc == MC - 1))
                    osb = wk_pool.tile([P, E], f32, tag="osb")
                    nc.vector.tensor_copy(osb[:], ps[:])
                    nc.sync.dma_start(outd[t * P:(t + 1) * P, :], osb[:])

            if rep_ctx is not None:
                rep_ctx.__exit__(None, None, None)

    _split_sync_waits(nc)
    return nc


def _get_nc(needs_bv: bool, reps: int = 1):
    key = ("nc", needs_bv, reps)
    if key not in _CACHE:
        _CACHE[key] = _build(needs_bv, reps)
    return _CACHE[key]


def kernel(query, key_value, kv_mask, sparse_mask,
           ln_q_g, ln_q_b, ln_kv_g, ln_kv_b,
           Wq, bq, Wk, bk, Wv, bv, Wo, bo):
    query = np.asarray(query, np.float32)
    key_value = np.asarray(key_value, np.float32)
    kv_mask = np.asarray(kv_mask)
    sparse_mask = np.asarray(sparse_mask)
    B = query.shape[0]

    # Fold LN gain/bias into the projection weights (exact algebra):
    # (x_ln*g + b) @ W + c  ==  x_ln @ (g[:,None]*W) + (b@W + c)
    Wq_g = np.asarray(ln_q_g, np.float32)[:, None] * np.asarray(Wq, np.float32)
    Wk_g = np.asarray(ln_kv_g, np.float32)[:, None] * np.asarray(Wk, np.float32)
    Wv_g = np.asarray(ln_kv_g, np.float32)[:, None] * np.asarray(Wv, np.float32)
    bq_e = np.asarray(ln_q_b, np.float32) @ np.asarray(Wq, np.float32) + bq
    bk_e = np.asarray(ln_kv_b, np.float32) @ np.asarray(Wk, np.float32) + bk
    bv_e = np.asarray(ln_kv_b, np.float32) @ np.asarray(Wv, np.float32) + bv

    needs_bv = bool(np.any(bv_e != 0.0))
    reps = int(os.environ.get("KERNEL_REPS", "1"))
    nc = _get_nc(needs_bv, reps)

    in_maps = []
    for c in range(8):
        b, hg = c // 2, c % 2
        hs = slice(hg * MC * P, (hg + 1) * MC * P)
        m = {
            "xq": np.ascontiguousarray(query[b]),
            "xkv": np.ascontiguousarray(key_value[b]),
            "wq": np.ascontiguousarray(Wq_g[:, hs]).astype(BF16),
            "wk": np.ascontiguousarray(Wk_g[:, hs]).astype(BF16),
            "wv": np.ascontiguousarray(Wv_g[:, hs]).astype(BF16),
            "wo": np.ascontiguousarray(np.asarray(Wo, np.float32)[hs, :]).astype(BF16),
            "bq": np.ascontiguousarray(bq_e[hs].reshape(MC, P).T),
            "bk": np.ascontiguousarray(bk_e[hs].reshape(MC, P).T),
            "kvm": np.ascontiguousarray(
                kv_mask[b].astype(np.float32).reshape(NT, P).T),
            "mt": np.ascontiguousarray(sparse_mask[b].T).astype(BF16),
        }
        if needs_bv:
            m["bv"] = bv_e[hs].astype(BF16).reshape(1, MC * P)
        in_maps.append(m)

    res = bass_utils.run_bass_kernel_spmd(
        nc, in_maps, core_ids=list(range(8)),
        trace=bool(os.environ.get("KERNEL_TRACE")))
    globals()["LAST_RESULTS"] = res

    bo_f = np.asarray(bo, np.float32)
    out = np.empty((B, T, E), np.float32)
    for b in range(B):
        out[b] = res.results[2 * b]["out"] + res.results[2 * b + 1]["out"] + bo_f
    return out


# revision 50
# speedup vs baseline: 1.2679x; 1.0297x over previous
"""Trainium2 Bass kernel for nn_CrossAttentionLayer (sparse cross attention).

Sharding: 8 cores = 4 batches x 2 head-groups. Core c handles batch c//2 and
heads [4*(c%2), 4*(c%2)+4). Each core computes LN + q/k/v projections for its
shard, flash-style masked attention in transposed layout, and a partial
out-projection. Host sums the two per-batch partials and adds bo.

Device algorithm (per core), all matmuls bf16 with fp32 PSUM accumulation:
  xlnT   = transpose(layernorm(x))            (LN gains/biases folded into W/b)
  qT/kT  = W.T @ xlnT   [d, tok]              (per-partition bias via ACT)
  v      = xlnT.T @ Wv  [tok, d]  * kv_mask   (kv_mask folded into v + ones col)
  sT     = kT.T-blocks @ qT-blocks            [k, q] scores, transposed
  pT     = exp(sT * scale) * sparse_mask.T    (ACT exp + DVE mask multiply)
  accT   = [v | kvm].T @ pT                   rows 0-63 = unnormalized out.T,
                                              row 64 = softmax denominator
  aT     = accT[0:64] * (1/denominator)       (broadcast via DMA replicate)
  out    = aT.T-blocks @ Wo-blocks            [q, E] partial, fp32 to HBM
"""

import os

import numpy as np
import ml_dtypes

import bass_rust
import concourse.bass as bass
import concourse.mybir as mybir
import concourse.tile as tile
from concourse import bass_utils
from concourse.masks import make_identity
from concourse.vector_clock import ScopedClock


class _TileContext(tile.TileContext):
    """TileContext whose kernel-tail drain is split into single-wait drains.

    The walrus build in this environment rejects >1 sync-wait on a Drain
    (CTRL_NO struct): "Too many sync wait commands". The stock
    _drain_and_barrier attaches one wait per outstanding semaphore to a
    single Drain; emit one Drain per wait instead.
    """

    def _drain_and_barrier(self, tick_clock, wait_clock):
        drain_inst = self.nc.sync.drain()
        wait_clock.add_sem_waits(
            drain_inst.ins, ScopedClock({None: tick_clock.global_clock})
        )
        si = drain_inst.ins.sync_info
        if si is not None and si.on_wait and len(si.on_wait) > 1:
            waits = list(si.on_wait)
            drain_inst.ins.sync_info = bass_rust.SyncInfo(
                on_wait=[waits[0]], on_update=si.on_update or [])
            for w in waits[1:]:
                extra = self.nc.sync.drain()
                extra.ins.sync_info = bass_rust.SyncInfo(
                    on_wait=[w], on_update=[])

        self.nc.all_engine_barrier()
        assert self.sems is not None
        popped = self.nc._tile_sem_poison_stack.pop()
        assert popped is self._sem_poison
        self.nc.clear_and_free_semaphores(list(self.sems.allocated().values()))
        self.nc.all_engine_barrier()

def _split_sync_waits(nc):
    """Cap every instruction at one sync wait.

    This walrus build rejects instructions carrying more than one sem wait
    ("Too many sync wait commands", setupSyncWait) across several structs
    (Drain, DMACopy, ...). Move excess waits onto no-op instructions placed
    immediately before the offender on the same engine — identical ordering
    semantics, one wait per instruction.
    """
    for f in nc.m.functions:
        for bb in f.blocks:
            insns = bb.instructions
            out = []
            changed = False
            for ins in insns:
                si = ins.sync_info
                if si is not None and si.on_wait and len(si.on_wait) > 1:
                    waits = list(si.on_wait)
                    for w in waits[:-1]:
                        nop = mybir.InstNoOp(
                            name=nc.get_next_instruction_name(),
                            engine=ins.engine,
                            ins=[], outs=[],
                            sync_info=bass_rust.SyncInfo(
                                on_wait=[w], on_update=[]),
                        )
                        out.append(nop)
                    ins.sync_info = bass_rust.SyncInfo(
                        on_wait=[waits[-1]], on_update=si.on_update or [])
                    changed = True
                out.append(ins)
            if changed:
                bb.instructions = out


BF16 = ml_dtypes.bfloat16

E = 512
H = 8
D = 64
T = 2048           # tokens (both query and key side)
P = 128
NT = T // P        # 16 token tiles
EC = E // P        # 4 contraction chunks
HC = 4             # heads per core
MC = 2             # 128-wide chunks of this core's 256 head dims
QC = 4             # 512-wide query chunks
SCALE = float(D) ** -0.5
EPS = 1e-5

_CACHE = {}


def _build(needs_bv: bool, reps: int = 1):
    nc = bass.Bass("TRN2", target_bir_lowering=False, debug=False, num_devices=8)
    f32 = mybir.dt.float32
    bf16 = mybir.dt.bfloat16

    xq = nc.dram_tensor("xq", [T, E], f32, kind="ExternalInput").ap()
    xkv = nc.dram_tensor("xkv", [T, E], f32, kind="ExternalInput").ap()
    wq = nc.dram_tensor("wq", [E, MC * P], bf16, kind="ExternalInput").ap()
    wk = nc.dram_tensor("wk", [E, MC * P], bf16, kind="ExternalInput").ap()
    wv = nc.dram_tensor("wv", [E, MC * P], bf16, kind="ExternalInput").ap()
    wo = nc.dram_tensor("wo", [MC * P, E], bf16, kind="ExternalInput").ap()
    bqd = nc.dram_tensor("bq", [P, MC], f32, kind="ExternalInput").ap()
    bkd = nc.dram_tensor("bk", [P, MC], f32, kind="ExternalInput").ap()
    kvmd = nc.dram_tensor("kvm", [P, NT], f32, kind="ExternalInput").ap()
    mtd = nc.dram_tensor("mt", [T, T], bf16, kind="ExternalInput").ap()
    if needs_bv:
        bvd = nc.dram_tensor("bv", [1, MC * P], bf16, kind="ExternalInput").ap()
    outd = nc.dram_tensor("out", [T, E], f32, kind="ExternalOutput").ap()

    with _TileContext(nc) as tc:
        with (
            tc.tile_pool(name="persist", bufs=1) as pp,
            tc.tile_pool(name="xs", bufs=5) as xpool,
            tc.tile_pool(name="work", bufs=5) as wk_pool,
            tc.tile_pool(name="scratch", bufs=4) as scratch,
            tc.tile_pool(name="psA", bufs=2, space="PSUM") as psA,
            tc.tile_pool(name="psS", bufs=2, space="PSUM") as psS,
            tc.tile_pool(name="psC", bufs=2, space="PSUM") as psC,
        ):
            # ---- persistent SBUF tensors ----
            # xlnT / qT / aT are split into per-group tensors so downstream
            # consumers unblock as soon as their group is written (Tile tracks
            # dependencies per tile, so monolithic tensors serialize phases).
            wq_sb = pp.tile([P, EC, MC * P], bf16, tag="wq")
            wk_sb = pp.tile([P, EC, MC * P], bf16, tag="wk")
            wv_sb = pp.tile([P, EC, MC * P], bf16, tag="wv")
            wo_sb = pp.tile([P, MC, E], bf16, tag="wo")
            bq_sb = pp.tile([P, MC], f32, tag="bq")
            bk_sb = pp.tile([P, MC], f32, tag="bk")
            kvm_sb = pp.tile([P, NT], f32, tag="kvm")
            mt_gt = [pp.tile([P, 4, T], bf16, tag=f"mt{g}", name=f"mt{g}")
                     for g in range(4)]
            xlnq_g = [pp.tile([P, 4, EC, P], bf16, tag=f"xlnq{g}",
                              name=f"xlnq{g}") for g in range(4)]
            xlnkv_g = [pp.tile([P, 4, EC, P], bf16, tag=f"xlnkv{g}",
                               name=f"xlnkv{g}") for g in range(4)]
            qT_g = [pp.tile([P, MC, 512], bf16, tag=f"qT{g}", name=f"qT{g}")
                    for g in range(4)]
            kT_gt = [pp.tile([P, MC, 512], bf16, tag=f"kT{g}", name=f"kT{g}")
                     for g in range(4)]
            v_gt = [pp.tile([P, 4, HC * (D + 1)], bf16, tag=f"v{g}",
                            name=f"v{g}") for g in range(4)]
            aT_g = [pp.tile([P, MC, 512], bf16, tag=f"aT{g}", name=f"aT{g}")
                    for g in range(4)]
            sums_g = [pp.tile([P, HC * 4], f32, tag=f"sums{g}",
                              name=f"sums{g}") for g in range(QC)]
            rsp_g = [pp.tile([P, HC * 4], bf16, tag=f"rsp{g}",
                             name=f"rsp{g}") for g in range(QC)]
            rs_flat_g = [pp.tile([1, HC, 512], bf16, tag=f"rsflat{g}",
                                 name=f"rsflat{g}") for g in range(QC)]
            if needs_bv:
                bv_sb = pp.tile([1, MC * P], bf16, tag="bv")
                ones_sb = pp.tile([1, P], bf16, tag="ones")

            eps_sb = pp.tile([P, 1], f32, tag="eps")
            nc.vector.memset(eps_sb[:], EPS)
            if needs_bv:
                nc.sync.dma_start(bv_sb[:], bvd)
                nc.vector.memset(ones_sb[:], 1.0)
            ident = pp.tile([P, P], bf16, tag="ident")
            make_identity(nc, ident[:])
            ones1 = pp.tile([1, D], bf16, tag="ones1")
            nc.vector.memset(ones1[:], 1.0)
            mtr = mtd.rearrange("(c p) q -> p c q", p=P)

            def ln_group(src, dstT, g):
                """LN 4 token tiles of src into dstT ([P, 4, EC, P])."""
                for tp in range(2):
                    # one [128, 1024] bf16 psum tile (1 bank) holds the
                    # transposes of a PAIR of token tiles; one wide copy out
                    ptr = psA.tile([P, 2, E], bf16, tag="p512")
                    for ti2 in range(2):
                        ti = tp * 2 + ti2
                        t = g * 4 + ti
                        xt = xpool.tile([P, E], f32, tag="x")
                        nc.sync.dma_start(xt[:], src[t * P:(t + 1) * P, :])
                        stats = scratch.tile([P, 6], f32, tag="bnstats")
                        mv = scratch.tile([P, 2], f32, tag="bnmv")
                        nc.vector.bn_stats(stats[:], xt[:])
                        nc.vector.bn_aggr(mv[:], stats[:])
                        sig = scratch.tile([P, 1], f32, tag="sig")
                        nc.scalar.activation(
                            sig[:], mv[:, 1:2],
                            mybir.ActivationFunctionType.Sqrt,
                            bias=eps_sb[:])
                        rsig = scratch.tile([P, 1], f32, tag="rsig")
                        nc.vector.reciprocal(rsig[:], sig[:])
                        xln = wk_pool.tile([P, E], bf16, tag="xln")
                        nc.vector.tensor_scalar(
                            xln[:], xt[:], mv[:, 0:1], rsig[:],
                            mybir.AluOpType.subtract, mybir.AluOpType.mult)
                        for c in range(EC):
                            nc.tensor.transpose(
                                ptr[:, ti2, c * P:(c + 1) * P],
                                xln[:, c * P:(c + 1) * P], ident[:])
                    nc.vector.tensor_copy(
                        dstT[:, 2 * tp:2 * tp + 2],
                        ptr[:].rearrange("p u (c n) -> p u c n", n=P))

            def kproj_group(g):
                for mc in range(MC):
                    ps = psA.tile([P, 512], mybir.dt.float32, tag="p512")
                    for c in range(EC):
                        nc.tensor.matmul(
                            ps[:],
                            lhsT=wk_sb[:, c, mc * P:(mc + 1) * P],
                            rhs=xlnkv_g[g][:, :, c, :],
                            start=(c == 0), stop=(c == EC - 1))
                    nc.scalar.activation(
                        kT_gt[g][:, mc, :], ps[:],
                        mybir.ActivationFunctionType.Identity,
                        bias=bk_sb[:, mc:mc + 1])

            def vproj_group(g):
                for ti in range(4):
                    t = g * 4 + ti
                    ps = psA.tile([P, MC * P], mybir.dt.float32, tag="p512")
                    for c in range(EC):
                        nc.tensor.matmul(
                            ps[:],
                            lhsT=xlnkv_g[g][:, ti, c, :],
                            rhs=wv_sb[:, c, :],
                            start=(c == 0),
                            stop=(c == EC - 1 and not needs_bv))
                    if needs_bv:
                        nc.tensor.matmul(
                            ps[:], lhsT=ones_sb[:], rhs=bv_sb[:],
                            start=False, stop=True)
                    vd = v_gt[g][:, ti].rearrange("p (h d) -> p h d", d=D + 1)
                    nc.vector.tensor_scalar(
                        vd[:, :, 0:D], ps.rearrange("p (h d) -> p h d", d=D),
                        kvm_sb[:, t:t + 1], None, mybir.AluOpType.mult)
                    nc.vector.tensor_copy(
                        vd[:, :, D], kvm_sb[:, t:t + 1].to_broadcast((P, HC)))

            def qproj_group(g):
                for mc in range(MC):
                    ps = psA.tile([P, 512], mybir.dt.float32, tag="p512")
                    for c in range(EC):
                        nc.tensor.matmul(
                            ps[:],
                            lhsT=wq_sb[:, c, mc * P:(mc + 1) * P],
                            rhs=xlnq_g[g][:, :, c, :],
                            start=(c == 0), stop=(c == EC - 1))
                    nc.scalar.activation(
                        qT_g[g][:, mc, :], ps[:],
                        mybir.ActivationFunctionType.Identity,
                        bias=bq_sb[:, mc:mc + 1])

            # kv side first (attention needs all of kT/v); mask chunks stream
            # in behind the x loads; q-side groups unblock attention per qc.
            rep_ctx = tc.For_i(0, reps, 1) if reps > 1 else None
            if rep_ctx is not None:
                rep_ctx.__enter__()
            for g in range(4):
                ln_group(xkv, xlnkv_g[g], g)
                if g == 0:
                    # weights land behind the first x tiles on the DMA queues
                    nc.sync.dma_start(
                        wk_sb[:], wk.rearrange("(c p) n -> p c n", p=P))
                    nc.sync.dma_start(
                        wv_sb[:], wv.rearrange("(c p) n -> p c n", p=P))
                    nc.sync.dma_start(
                        wq_sb[:], wq.rearrange("(c p) n -> p c n", p=P))
                    nc.sync.dma_start(
                        wo_sb[:], wo.rearrange("(c p) n -> p c n", p=P))
                    nc.sync.dma_start(bq_sb[:], bqd)
                    nc.sync.dma_start(bk_sb[:], bkd)
                    nc.sync.dma_start(kvm_sb[:], kvmd)
                kproj_group(g)
                vproj_group(g)
                for c in range(4):
                    nc.sync.dma_start(mt_gt[g][:, c], mtr[:, 4 * g + c])
                if g == 1:
                    # q group 0 early: attention (qc=0) starts on k groups
                    # 0-1 while kv groups 2-3 are still in layernorm
                    ln_group(xq, xlnq_g[0], 0)
                    qproj_group(0)
            for g in range(1, 4):
                ln_group(xq, xlnq_g[g], g)
                qproj_group(g)

            # ---- attention: qc outer so normalize+out_proj overlap ----
            for qc in range(QC):
                for h in range(HC):
                    mc = h // 2
                    po = (h % 2) * D
                    acc = psC.tile([P, 512], mybir.dt.float32, tag="acc")
                    for kcp in range(NT // 2):
                        sp = psS.tile([P, 2, 512], mybir.dt.float32, tag="sp")
                        kg = kcp // 2          # k group (4 k-chunks each)
                        ko = (2 * kcp) % 4     # chunk offset inside group
                        for j in range(2):
                            nc.tensor.matmul(
                                sp[:, j],
                                lhsT=kT_gt[kg][po:po + D, mc,
                                               (ko + j) * P:(ko + j + 1) * P],
                                rhs=qT_g[qc][po:po + D, mc, :],
                                start=True, stop=True)
                        pT = wk_pool.tile([P, 2, 512], bf16, tag="pT")
                        nc.scalar.activation(
                            pT[:], sp[:], mybir.ActivationFunctionType.Exp,
                            scale=SCALE)
                        nc.vector.tensor_tensor(
                            pT[:], pT[:],
                            mt_gt[kg][:, ko:ko + 2, qc * 512:(qc + 1) * 512],
                            mybir.AluOpType.mult)
                        for j in range(2):
                            nc.tensor.matmul(
                                acc[:D + 1],
                                lhsT=v_gt[kg][:, ko + j,
                                              h * (D + 1):(h + 1) * (D + 1)],
                                rhs=pT[:, j],
                                start=(kcp == 0 and j == 0),
                                stop=(kcp == NT // 2 - 1 and j == 1))
                    stage = wk_pool.tile([P, 512], f32, tag="sumstage")
                    nc.vector.tensor_copy(stage[D:D + 1, :], acc[D:D + 1, :])
                    # gather this head's denominators into [P, 4] of sums_g
                    # (DMA streams element-linearly: q index = p*4 + f)
                    nc.sync.dma_start(
                        sums_g[qc][:, h * 4:(h + 1) * 4], stage[D:D + 1, :])
                    nc.vector.tensor_copy(
                        aT_g[qc][po:po + D, mc, :], acc[:D])

                    if h % 2 == 1:
                        # this head pair (chunk mc) is complete: normalize it
                        # now so the chain overlaps the remaining heads
                        sl = slice(8 * mc, 8 * mc + 8)
                        nc.vector.tensor_scalar(
                            sums_g[qc][:, sl], sums_g[qc][:, sl], 1e-30, None,
                            mybir.AluOpType.add)
                        rsp = rsp_g[qc]
                        rsf = scratch.tile([P, 8], f32, tag="rsf")
                        nc.vector.reciprocal(rsf[:], sums_g[qc][:, sl])
                        nc.vector.tensor_copy(rsp[:, sl], rsf[:])
                        for hh in (2 * mc, 2 * mc + 1):
                            nc.sync.dma_start(
                                rs_flat_g[qc][:, hh, :],
                                rsp[:, hh * 4:(hh + 1) * 4])
                        rb = psA.tile([P, 512], mybir.dt.float32, tag="p512")
                        nc.tensor.matmul(
                            rb[0:D], lhsT=ones1[:],
                            rhs=rs_flat_g[qc][:, 2 * mc, :],
                            start=True, stop=True)
                        nc.tensor.matmul(
                            rb[D:2 * D], lhsT=ones1[:],
                            rhs=rs_flat_g[qc][:, 2 * mc + 1, :],
                            start=True, stop=True)
                        nc.vector.tensor_tensor(
                            aT_g[qc][:, mc, :], aT_g[qc][:, mc, :],
                            rb[:], mybir.AluOpType.mult)

                # out projection for this qc's 4 token tiles
                for ti in range(4):
                    t = qc * 4 + ti
                    ps = psA.tile([P, E], mybir.dt.float32, tag="p512")
                    for mc in range(MC):
                        nc.tensor.matmul(
                            ps[:],
                            lhsT=aT_g[qc][:, mc, ti * P:(ti + 1) * P],
                            rhs=wo_sb[:, mc, :],
                            start=(mc == 0), stop=(mc == MC - 1))
                    osb = wk_pool.tile([P, E], f32, tag="osb")
                    nc.vector.tensor_copy(osb[:], ps[:])
                    nc.sync.dma_start(outd[t * P:(t + 1) * P, :], osb[:])

            if rep_ctx is not None:
                rep_ctx.__exit__(None, None, None)

    _split_sync_waits(nc)
    return nc


def _get_nc(needs_bv: bool, reps: int = 1):
    key = ("nc", needs_bv, reps)
    if key not in _CACHE:
        _CACHE[key] = _build(needs_bv, reps)
    return _CACHE[key]


def kernel(query, key_value, kv_mask, sparse_mask,
           ln_q_g, ln_q_b, ln_kv_g, ln_kv_b,
           Wq, bq, Wk, bk, Wv, bv, Wo, bo):
    query = np.asarray(query, np.float32)
    key_value = np.asarray(key_value, np.float32)
    kv_mask = np.asarray(kv_mask)
    sparse_mask = np.asarray(sparse_mask)
    B = query.shape[0]

    # Fold LN gain/bias into the projection weights (exact algebra):
    # (x_ln*g + b) @ W + c  ==  x_ln @ (g[:,None]*W) + (b@W + c)
    Wq_g = np.asarray(ln_q_g, np.float32)[:, None] * np.asarray(Wq, np.float32)
    Wk_g = np.asarray(ln_kv_g, np.float32)[:, None] * np.asarray(Wk, np.float32)
    Wv_g = np.asarray(ln_kv_g, np.float32)[:, None] * np.asarray(Wv, np.float32)
    bq_e = np.asarray(ln_q_b, np.float32) @ np.asarray(Wq, np.float32) + bq
    bk_e = np.asarray(ln_kv_b, np.float32) @ np.asarray(Wk, np.float32) + bk
    bv_e = np.asarray(ln_kv_b, np.float32) @ np.asarray(Wv, np.float32) + bv

    needs_bv = bool(np.any(bv_e != 0.0))
    reps = int(os.environ.get("KERNEL_REPS", "1"))
    nc = _get_nc(needs_bv, reps)

    in_maps = []
    for c in range(8):
        b, hg = c // 2, c % 2
        hs = slice(hg * MC * P, (hg + 1) * MC * P)
        m = {
            "xq": np.ascontiguousarray(query[b]),
            "xkv": np.ascontiguousarray(key_value[b]),
            "wq": np.ascontiguousarray(Wq_g[:, hs]).astype(BF16),
            "wk": np.ascontiguousarray(Wk_g[:, hs]).astype(BF16),
            "wv": np.ascontiguousarray(Wv_g[:, hs]).astype(BF16),
            "wo": np.ascontiguousarray(np.asarray(Wo, np.float32)[hs, :]).astype(BF16),
            "bq": np.ascontiguousarray(bq_e[hs].reshape(MC, P).T),
            "bk": np.ascontiguousarray(bk_e[hs].reshape(MC, P).T),
            "kvm": np.ascontiguousarray(
                kv_mask[b].astype(np.float32).reshape(NT, P).T),
            "mt": np.ascontiguousarray(sparse_mask[b].T).astype(BF16),
        }
        if needs_bv:
            m["bv"] = bv_e[hs].astype(BF16).reshape(1, MC * P)
        in_maps.append(m)

    res = bass_utils.run_bass_kernel_spmd(
        nc, in_maps, core_ids=list(range(8)),
        trace=bool(os.environ.get("KERNEL_TRACE")))
    globals()["LAST_RESULTS"] = res

    bo_f = np.asarray(bo, np.float32)
    out = np.empty((B, T, E), np.float32)
    for b in range(B):
        out[b] = res.results[2 * b]["out"] + res.results[2 * b + 1]["out"] + bo_f
    return out
